# revision 5
# baseline (speedup 1.0000x reference)
"""Single-head attention (B=4, S=4096, D=512, H=64) on 8 TRN2 NeuronCores, v2.

Sharding (same as v1): core c handles batch b=c//2 and key-half h=c%2 (2048
keys) for ALL 4096 queries; host merges the two halves per batch.

v2 speedups over v1:
- Probabilities quantized to fp8 e4m3 directly by the ACT exp (out dtype f8),
  attn matmul runs as fp8 DoubleRow over 256-key pairs (~2.4x per pair).
- A fraction of chunk-pairs (DVE_PAIRS) computes exp on the *vector* engine
  via a Schraudolph bit-trick: i16 = round(s*A + B) whose bits ARE the f16
  exp(s/8 + shift) (one tensor_scalar per pair-tile); those pairs run f16 attn.
  This splits the softmax-exp work across ACT and DVE, the joint bottleneck.
- Class-mean bias calibration: ACT->f8 conversion truncates (measured), the
  Schraudolph sawtooth over-estimates; per-class constant shifts (folded into
  the exp bias and the B constant) re-center both so the softmax merge stays
  unbiased. v16 is pre-scaled by exp(SHIFT_A - SHIFT_B) for consistency.
"""

import numpy as np

import concourse.bass as bass
import concourse.tile as tile
import concourse.mybir as mybir
from concourse import bass_utils

B, S, D, H = 4, 4096, 512, 64
N_CORES = 8
KC = S // 2          # keys per core
NCHUNK = KC // 128   # 16 key chunks of 128
NPAIR = NCHUNK // 2  # 8 chunk pairs
MT = 512             # query tile width
NM = S // MT         # 8 query tiles
VW = 96              # v columns (64 v + 1 ones + 31 pad; DoubleRow needs %32)
NDC = D // 128       # 4 contraction chunks

DVE_PAIRS = (1, 4, 6)  # pair indices whose exp runs on DVE (f16 path)
PO_ON_ACT = False
WIDE_BIAS = True       # q/k bias adds over 1024-wide psum tiles

L2E = float(np.log2(np.e))
SHIFT_A = -3.2035    # fp8 class shift (incl. +0.0465 trunc-bias calibration)
SHIFT_B = -2.0       # f16/Schraudolph class shift
SCH_CORR = 59.0      # Schraudolph sawtooth re-centering, in i16 bits
A_SCH = 0.125 * L2E * 1024.0
B_SCH = 15360.0 - SCH_CORR + SHIFT_B * L2E * 1024.0
SB_SCALE = float(np.exp(SHIFT_A - SHIFT_B))  # v16 pre-scale

f16 = mybir.dt.float16
f32 = mybir.dt.float32
f8 = mybir.dt.float8e4
i16 = mybir.dt.int16


_SELF_CLOCK = {
    "EngineType.Activation": "Activation_",
    "EngineType.DVE": "DVE_",
    "EngineType.PE": "PE_",
}
_DROP_SELF_WAITS = [True]


def _split_multi_waits(nc):
    """This walrus build encodes at most ONE sync-wait command per
    instruction. First drop same-engine clock waits (trivially satisfied on
    an in-order serially-executing engine), then hoist surplus waits onto
    standalone single-wait EventSemaphore instructions."""
    ctr = 0
    for f in nc.m.functions:
        for bb in f.blocks:
            insts = bb.instructions
            i = 0
            while i < len(insts):
                inst = insts[i]
                si = inst.sync_info
                if si is not None and si.on_wait and len(si.on_wait) > 1:
                    pfx = _SELF_CLOCK.get(str(inst.engine)) if _DROP_SELF_WAITS[0] else None
                    waits = list(si.on_wait)
                    if pfx is not None:
                        kept = [
                            w for w in waits
                            if not (w.ant_name or "").startswith(pfx)
                        ]
                        if kept:
                            waits = kept
                    if len(waits) == 1:
                        inst.sync_info = mybir.SyncInfo(
                            on_wait=waits, on_update=list(si.on_update or [])
                        )
                        i += 1
                        continue
                    inst.sync_info = mybir.SyncInfo(
                        on_wait=[waits[-1]], on_update=list(si.on_update or [])
                    )
                    for w in waits[:-1]:
                        ev = mybir.InstEventSemaphore(
                            name=f"W-split-{ctr}", ins=[], outs=[]
                        )
                        ctr += 1
                        ev.engine = inst.engine
                        ev.sync_info = mybir.SyncInfo(on_wait=[w], on_update=[])
                        insts.insert(i, ev)
                        i += 1
                i += 1
    return ctr


def _trim_tail_barrier(nc):
    """Drop the second all-engine barrier after the tail sem-clear (saves
    ~2 us of end-of-kernel EVSEM butterfly)."""
    for f in nc.m.functions:
        for bb in f.blocks:
            if not bb.name.endswith("_end"):
                continue
            insts = bb.instructions
            last_isa = None
            for i, inst in enumerate(insts):
                if type(inst).__name__ == "InstISA":
                    last_isa = i
            if last_isa is not None:
                while len(insts) > last_isa + 1:
                    insts.pop()


def _build_nc(reps=1, phase="full"):
    nc = bass.Bass("TRN2", target_bir_lowering=False, debug=False)
    xt = nc.dram_tensor("xt", [D, S], f16, kind="ExternalInput").ap()
    wq = nc.dram_tensor("wq", [D, 128], f16, kind="ExternalInput").ap()
    wk = nc.dram_tensor("wk", [D, 128], f16, kind="ExternalInput").ap()
    wv = nc.dram_tensor("wv", [D, VW], f16, kind="ExternalInput").ap()
    bq = nc.dram_tensor("bq", [128, 1], f32, kind="ExternalInput").ap()
    bk = nc.dram_tensor("bk", [128, 1], f32, kind="ExternalInput").ap()
    bv8 = nc.dram_tensor("bv8", [1, VW], f32, kind="ExternalInput").ap()
    bvB = nc.dram_tensor("bvB", [1, 65], f32, kind="ExternalInput").ap()
    out = nc.dram_tensor("out", [65, S], f32, kind="ExternalOutput").ap()

    def body(tc, rep, consts, xsb, qkv, ptp, outsb, wq_sb, wk_sb, wv_sb,
             bq_sb, bk_sb, bv8_sb, bvB_sb, shiftA_sb):
        # --- x^T load in query-column blocks ---
        x_sb = xsb.tile([128, NDC, S], f16, tag="x")
        xt_r = xt.rearrange("(c p) s -> p c s", p=128)
        for blk in range(NM):
            bs = blk * MT
            if blk == 0:
                for c in range(NDC):
                    nc.sync.dma_start(
                        out=x_sb[:, c, bs:bs + MT], in_=xt_r[:, c, bs:bs + MT]
                    )
            else:
                nc.sync.dma_start(
                    out=x_sb[:, :, bs:bs + MT], in_=xt_r[:, :, bs:bs + MT]
                )
        if phase == "dma":
            return

        # --- projections ---
        qTd_sb = qkv.tile([128, S], f16, tag="qT")      # q^T dup'd on 64..127
        kTd_sb = qkv.tile([128, KC], f16, tag="kT")     # k^T dup'd
        v8_sb = qkv.tile([128, NCHUNK, VW], f8, tag="v8")   # [v|1|0] fp8
        v16_sb = qkv.tile([128, NCHUNK, 65], f16, tag="v16")  # [v|1]*sB f16

        dve_chunks = set()
        for j in DVE_PAIRS:
            dve_chunks.add(2 * j)
            dve_chunks.add(2 * j + 1)

        PB = 2 * MT if WIDE_BIAS else MT  # bias-add tile width
        with tc.tile_pool(name=f"pproj{rep}", bufs=3, space="PSUM") as pproj:
            for blk in range(NM):
                bs = blk * MT
                if blk < KC // MT:  # key blocks: k^T then v
                    if not WIDE_BIAS or blk % 2 == 0:
                        psk = pproj.tile([128, PB], f32, tag="pq",
                                         bufs=2 if WIDE_BIAS else 3)
                    ko = (blk % 2) * MT if WIDE_BIAS else 0
                    for c in range(NDC):
                        nc.tensor.matmul(
                            psk[:, ko:ko + MT], lhsT=wk_sb[:, c, :],
                            rhs=x_sb[:, c, bs:bs + MT],
                            start=(c == 0), stop=(c == NDC - 1),
                        )
                    if not WIDE_BIAS or blk % 2 == 1:
                        kb = (bs - MT) if WIDE_BIAS else bs
                        nc.vector.tensor_scalar_add(
                            out=kTd_sb[:, kb:kb + PB], in0=psk, scalar1=bk_sb
                        )
                if not WIDE_BIAS or blk % 2 == 0:
                    psq = pproj.tile([128, PB], f32, tag="pq",
                                     bufs=2 if WIDE_BIAS else 3)
                qo = (blk % 2) * MT if WIDE_BIAS else 0
                for c in range(NDC):
                    nc.tensor.matmul(
                        psq[:, qo:qo + MT], lhsT=wq_sb[:, c, :],
                        rhs=x_sb[:, c, bs:bs + MT],
                        start=(c == 0), stop=(c == NDC - 1),
                    )
                if not WIDE_BIAS or blk % 2 == 1:
                    qb = (bs - MT) if WIDE_BIAS else bs
                    nc.vector.tensor_scalar_add(
                        out=qTd_sb[:, qb:qb + PB], in0=psq, scalar1=bq_sb
                    )
                if blk < KC // MT:
                    for s_ in range(4 * blk, 4 * blk + 4):  # v chunks
                        ps = pproj.tile([128, VW], f32, tag="pv")
                        for c in range(NDC):
                            nc.tensor.matmul(
                                ps, lhsT=x_sb[:, c, s_ * 128:(s_ + 1) * 128],
                                rhs=wv_sb[:, c, :],
                                start=(c == 0), stop=(c == NDC - 1),
                            )
                        nc.vector.tensor_add(
                            out=v8_sb[:, s_, :], in0=ps, in1=bv8_sb
                        )
                        if s_ in dve_chunks:
                            nc.vector.scalar_tensor_tensor(
                                out=v16_sb[:, s_, :], in0=ps[:, 0:65],
                                scalar=SB_SCALE, in1=bvB_sb,
                                op0=mybir.AluOpType.mult,
                                op1=mybir.AluOpType.add,
                            )
        if phase == "proj":
            return

        # --- main attention loop ---
        with (
            tc.tile_pool(name=f"psc{rep}", bufs=3, space="PSUM") as pscp,
            tc.tile_pool(name=f"pout{rep}", bufs=2, space="PSUM") as poutp,
        ):
            for m in range(NM):
                ms = m * MT
                po = poutp.tile([VW, MT], f32, tag="po")
                pending = []  # (kind, pt, j); attn lags scores by 2 pair-groups
                for j in range(NPAIR):
                    c0 = 2 * j
                    psc = pscp.tile([128, 2 * MT], f32, tag="psc")
                    for q in range(2):
                        ck = c0 + q
                        rb = 64 * (ck % 2)
                        nc.tensor.matmul(
                            psc[:, q * MT:(q + 1) * MT],
                            lhsT=kTd_sb[rb:rb + 64, ck * 128:(ck + 1) * 128],
                            rhs=qTd_sb[rb:rb + 64, ms:ms + MT],
                            start=True, stop=True,
                        )
                    if phase == "scores":
                        continue
                    if j in DVE_PAIRS:
                        ptb = ptp.tile([128, 2 * MT], i16, tag="ptb")
                        nc.vector.tensor_scalar(
                            out=ptb, in0=psc, scalar1=A_SCH, scalar2=B_SCH,
                            op0=mybir.AluOpType.mult, op1=mybir.AluOpType.add,
                        )
                        ent = ("f16", ptb, j)
                    else:
                        pt8 = ptp.tile([128, 2 * MT], f8, tag="pt8")
                        nc.scalar.activation(
                            out=pt8, in_=psc,
                            func=mybir.ActivationFunctionType.Exp,
                            bias=shiftA_sb, scale=0.125,
                        )
                        ent = ("f8", pt8, j)
                    if phase == "exp":
                        continue
                    pending.append(ent)
                    lag = 0 if m == NM - 1 else 2
                    if len(pending) > lag:
                        _emit_attn(nc, po, v8_sb, v16_sb, *pending.pop(0))
                if phase not in ("scores", "exp"):
                    for p in pending:
                        _emit_attn(nc, po, v8_sb, v16_sb, *p)
                    po_sb = outsb.tile([65, MT], f32, tag="posb")
                    # alternate the psum->sbuf copy between ACT and DVE so the
                    # two softmax engines stay balanced (Copy shares the exp
                    # table set, so no ACT table reload)
                    if PO_ON_ACT or m % 2 == 1:
                        nc.scalar.activation(
                            out=po_sb, in_=po[0:65, :],
                            func=mybir.ActivationFunctionType.Copy,
                            bias=0.0, scale=1.0,
                        )
                    else:
                        nc.vector.tensor_copy(out=po_sb, in_=po[0:65, :])
                    nc.sync.dma_start(out=out[:, m * MT:(m + 1) * MT], in_=po_sb)

    with tile.TileContext(nc) as tc:
        with (
            tc.tile_pool(name="consts", bufs=1) as consts,
            tc.tile_pool(name="xsb", bufs=1) as xsb,
            tc.tile_pool(name="qkv", bufs=2) as qkv,
            tc.tile_pool(name="pt", bufs=6) as ptp,
            tc.tile_pool(name="outsb", bufs=3) as outsb,
        ):
            wq_sb = consts.tile([128, NDC, 128], f16)
            wk_sb = consts.tile([128, NDC, 128], f16)
            wv_sb = consts.tile([128, NDC, VW], f16)
            nc.sync.dma_start(out=wq_sb, in_=wq.rearrange("(c p) m -> p c m", p=128))
            nc.sync.dma_start(out=wk_sb, in_=wk.rearrange("(c p) m -> p c m", p=128))
            nc.sync.dma_start(out=wv_sb, in_=wv.rearrange("(c p) m -> p c m", p=128))
            bq_sb = consts.tile([128, 1], f32)
            bk_sb = consts.tile([128, 1], f32)
            bv8_sb = consts.tile([128, VW], f32)
            bvB_sb = consts.tile([128, 65], f32)
            nc.sync.dma_start(out=bq_sb, in_=bq)
            nc.sync.dma_start(out=bk_sb, in_=bk)
            bv8_bcast = bass.AP(tensor=bv8.tensor, offset=bv8.offset, ap=[[0, 128], [1, VW]])
            nc.sync.dma_start(out=bv8_sb, in_=bv8_bcast)
            bvB_bcast = bass.AP(tensor=bvB.tensor, offset=bvB.offset, ap=[[0, 128], [1, 65]])
            nc.sync.dma_start(out=bvB_sb, in_=bvB_bcast)
            shiftA_sb = consts.tile([128, 1], f32)
            nc.vector.memset(shiftA_sb, SHIFT_A)

            for rep in range(reps):
                body(tc, rep, consts, xsb, qkv, ptp, outsb, wq_sb, wk_sb,
                     wv_sb, bq_sb, bk_sb, bv8_sb, bvB_sb, shiftA_sb)

    _split_multi_waits(nc)
    _trim_tail_barrier(nc)
    return nc


def _emit_attn(nc, po, v8_sb, v16_sb, kind, pt, j):
    c0 = 2 * j
    first = (j == 0)
    last = (j == NPAIR - 1)
    if kind == "f8":
        nc.tensor.matmul(
            po, lhsT=v8_sb[:, c0:c0 + 2, :],
            rhs=pt.rearrange("p (c m) -> p c m", c=2),
            start=first, stop=last,
            perf_mode=mybir.MatmulPerfMode.DoubleRow,
        )
    else:
        ptf = pt.bitcast(f16)
        for q in range(2):
            ck = c0 + q
            nc.tensor.matmul(
                po[0:65, :], lhsT=v16_sb[:, ck, :],
                rhs=ptf[:, q * MT:(q + 1) * MT],
                start=(first and q == 0), stop=(last and q == 1),
            )


_NC_CACHE = []


def _prepare_in_maps(x, Wq, bq, Wk, bk, Wv, bv):
    x = np.asarray(x, dtype=np.float32)
    Wq, Wk, Wv = (np.asarray(a, dtype=np.float32) for a in (Wq, Wk, Wv))
    bq, bk, bv = (np.asarray(a, dtype=np.float32) for a in (bq, bk, bv))

    wq_dup = np.concatenate([Wq, Wq], axis=1).astype(np.float16)      # [512,128]
    wk_dup = np.concatenate([Wk, Wk], axis=1).astype(np.float16)
    wv_aug = np.concatenate(
        [Wv, np.zeros((D, VW - H), np.float32)], axis=1).astype(np.float16)
    bq_dup = np.concatenate([bq, bq]).astype(np.float32).reshape(128, 1)
    bk_dup = np.concatenate([bk, bk]).astype(np.float32).reshape(128, 1)
    ones1 = np.ones(1, np.float32)
    bv8_aug = np.concatenate(
        [bv, ones1, np.zeros(VW - H - 1, np.float32)]).astype(np.float32).reshape(1, VW)
    bvB_aug = (np.concatenate([bv, ones1]) * SB_SCALE).astype(np.float32).reshape(1, 65)

    in_maps = []
    for c in range(N_CORES):
        b, h = c // 2, c % 2
        xt_b = np.ascontiguousarray(x[b].T)  # [512, 4096]
        if h == 1:
            xt_b = np.roll(xt_b, -KC, axis=1)
        in_maps.append({
            "xt": xt_b.astype(np.float16),
            "wq": wq_dup, "wk": wk_dup, "wv": wv_aug,
            "bq": bq_dup, "bk": bk_dup, "bv8": bv8_aug, "bvB": bvB_aug,
        })
    return in_maps


def _merge_results(results):
    out = np.empty((B, S, H), dtype=np.float32)
    for b in range(B):
        a = results[2 * b]["out"].astype(np.float64)
        bb = results[2 * b + 1]["out"].astype(np.float64)
        bb = np.roll(bb, KC, axis=1)
        tot = a + bb
        out[b] = (tot[:H, :] / tot[H:H + 1, :]).T.astype(np.float32)
    return out


def kernel(x, Wq, bq, Wk, bk, Wv, bv):
    in_maps = _prepare_in_maps(x, Wq, bq, Wk, bk, Wv, bv)
    if not _NC_CACHE:
        _NC_CACHE.append(_build_nc())
    nc = _NC_CACHE[0]
    res = bass_utils.run_bass_kernel_spmd(nc, in_maps, core_ids=list(range(N_CORES)))
    return _merge_results(res.results)
